# revision 25
# baseline (speedup 1.0000x reference)
"""Trainium2 Bass kernel for nn_Decoder (attention-conditioned GRU decoder step).

Strategy (data-parallel over batch B=128 across 8 cores, Bc=16 per core):
  - Host re-layouts inputs per core (transpose/cast only; all FLOPs on device):
      enc   -> enc_T tiles [n-chunk, ch-chunk, 128, 512] bf16 where the flat
               free index n = b*S + t (b-major) so softmax-over-t is chunk-local
      weights -> lhsT tile layout [128(p=k-in), kc, oc, 128(m)] bf16
  - Device per core:
      pass 1: pre^T[h, n] = u_a^T @ enc_T  (PE, bf16, fp32 PSUM)
              tanh(pre + q_b + bias) via ACT with per-partition bias
              scores = v_a . tanh  via PE (lhsT = v_a column)
              softmax per b (chunk-local, ACT exp + DVE reduce/reciprocal)
      pass 2: probs broadcast to 128 partitions via PE ones-row matmul,
              attended a^T[ch, b] via DVE multiply + segmented reduce
      GRU:    all gate matmuls as out^T[dout, b] = W^T(k-tiles) @ rhs^T,
              x/s-only partial sums run early (hidden under attention),
              a-dependent tail after pass 2. Pointwise on DVE, acts on ACT.
  - Embedding lookup on device via gpsimd indirect DMA gather.
"""

import os

os.environ.setdefault("MYCRO_LOCAL_CACHE", "1")

import numpy as np
import ml_dtypes

import concourse.bass as bass
import concourse.mybir as mybir
import concourse.tile as tile
from concourse import bacc
from concourse.bass_utils import run_bass_kernel_spmd
from concourse.masks import make_identity

BF16 = mybir.dt.bfloat16
F32 = mybir.dt.float32
I32 = mybir.dt.int32
nbf = ml_dtypes.bfloat16
AF = mybir.ActivationFunctionType
ALU = mybir.AluOpType

NCORES = 8
# full-problem dims (hardcoded per task contract)
V_FULL, E, H, S, B_FULL = 50000, 512, 1024, 256, 128
CH = 2 * H
P = 128
CHC = CH // P   # 16 ch chunks
HC = H // P     # 8 h chunks
EC = E // P     # 4 e chunks
CHUNK = 512     # free-dim chunk of n = b*S + t  (2 b's per chunk)
BPC = CHUNK // S  # b's per chunk = 2

GATE_SPECS = [
    # name, kc count, rhs source key
    ("wz_x", EC, "x"), ("wr_x", EC, "x"), ("w_x", EC, "x"), ("vo_x", EC, "x"),
    ("uz_s", HC, "s"), ("ur_s", HC, "s"),
    ("u_rs", HC, "rs"), ("uo_s", HC, "snew"),
    ("cz_a", CHC, "a"), ("cr_a", CHC, "a"), ("c_a", CHC, "a"), ("co_a", CHC, "a"),
]


def build_decoder_nc(Bc=16, V=V_FULL, enable_asserts=False):
    """Build + compile the per-core SPMD kernel. Returns the bacc object."""
    NCH = (Bc * S) // CHUNK  # n-chunks
    nc = bacc.Bacc(
        "TRN2", target_bir_lowering=False, debug=False, enable_asserts=enable_asserts
    )

    def din(name, shape, ty):
        return nc.dram_tensor(name, shape, ty, kind="ExternalInput").ap()

    def dout(name, shape, ty):
        return nc.dram_tensor(name, shape, ty, kind="ExternalOutput").ap()

    enc_t = din("enc_t", [NCH, CHC, P, CHUNK], BF16)
    ua_t = din("ua_t", [HC, P, CHC, P], BF16)
    wa_t = din("wa_t", [P, HC, HC, P], BF16)
    va_t = din("va_t", [P, HC], BF16)
    batt_t = din("batt_t", [P, HC], F32)
    s_t = din("s_t", [P, HC, Bc], F32)
    emb_t = din("emb_t", [V, E], BF16)
    iw_t = din("iw_t", [Bc, 1], I32)
    wts = {name: din(name, [P, kc, HC, P], BF16) for name, kc, _ in GATE_SPECS}
    bz_t = din("bz_t", [P, HC], F32)
    br_t = din("br_t", [P, HC], F32)
    bst_t = din("bst_t", [P, HC], F32)
    bt_t = din("bt_t", [P, HC], F32)

    probs_o = dout("probs_o", [NCH, CHUNK], F32)
    snew_o = dout("snew_o", [P, HC, Bc], F32)
    ti_o = dout("ti_o", [P, HC, Bc], F32)

    with tile.TileContext(nc) as tc:
        with (
            tc.tile_pool(name="const", bufs=1) as const,
            tc.tile_pool(name="stream", bufs=7) as stream,
            tc.tile_pool(name="stream4", bufs=4) as stream4,
            tc.tile_pool(name="scr", bufs=2) as scr,
            tc.tile_pool(name="pf", bufs=3) as pf_pool,
            tc.tile_pool(name="ps_attn", bufs=3, space="PSUM") as ps_attn,
            tc.tile_pool(name="ps_vsc", bufs=2, space="PSUM") as ps_vsc,
            tc.tile_pool(name="ps_bc", bufs=1, space="PSUM") as ps_bc,
            tc.tile_pool(name="ps_g", bufs=2, space="PSUM") as ps_g,
        ):
            # ---------------- constants / small inputs ----------------
            ident = const.tile([P, P], F32)
            make_identity(nc, ident[:])
            ones_row = const.tile([P, P], BF16)
            nc.vector.memset(ones_row[:], 0.0)
            nc.vector.memset(ones_row[0:1, :], 1.0)

            # sync (HWDGE) queue order is arrival-deadline order: iw (gather
            # gate), wa (q gate), ua+enc0 (first attention chunk), consts,
            # remaining enc chunks, then the a-dependent tail weights.
            iw_sb = const.tile([Bc, 1], I32)
            nc.sync.dma_start(iw_sb[:], iw_t[:])
            s_sb = const.tile([P, HC, Bc], F32)
            nc.sync.dma_start(s_sb[:], s_t[:])
            wa_sb = stream.tile([P, HC, HC, P], BF16, tag="big2m")
            nc.sync.dma_start(wa_sb[:], wa_t[:])
            batt_sb = const.tile([P, HC], F32)
            nc.sync.dma_start(batt_sb[:], batt_t[:])
            # u_a loads per-hc so chunk0/hc0 can start after only 1/8 of it
            ua_sb = const.tile([P, HC, CHC, P], BF16)
            nc.sync.dma_start(ua_sb[:, 0], ua_t[0])

            enc_tiles = []
            for j in range(NCH):
                ej = stream.tile([P, CHC, CHUNK], BF16, tag="big2m",
                                 name=f"enc_{j}")
                enc_tiles.append(ej)
            nc.sync.dma_start(enc_tiles[0][:],
                              enc_t[0].rearrange("c p n -> p c n"))
            for hc in range(1, HC):
                nc.sync.dma_start(ua_sb[:, hc], ua_t[hc])

            va_sb = const.tile([P, HC], BF16)
            nc.sync.dma_start(va_sb[:], va_t[:])
            bias_sb = {}
            for nm, t in (("z", bz_t), ("r", br_t), ("st", bst_t), ("t", bt_t)):
                bias_sb[nm] = const.tile([P, HC], F32, name=f"bias_{nm}")
                nc.sync.dma_start(bias_sb[nm][:], t[:])

            s_bf = const.tile([P, HC, Bc], BF16)
            nc.vector.tensor_copy(s_bf[:], s_sb[:])

            # embedding gather rides the gpsimd dynamic queue
            x_nat = const.tile([Bc, E], BF16)
            nc.gpsimd.indirect_dma_start(
                out=x_nat[:],
                out_offset=None,
                in_=emb_t[:],
                in_offset=bass.IndirectOffsetOnAxis(ap=iw_sb[:, :1], axis=0),
            )

            # ---------------- weight stream helpers ----------------
            def load_w(name, queue=None):
                kcn = dict((n, k) for n, k, _ in GATE_SPECS)[name]
                dram = wts[name]
                dma = (queue or nc.sync).dma_start
                tiles = []
                if kcn <= HC:
                    t = (stream if kcn == HC else stream4).tile(
                        [P, kcn, HC, P], BF16,
                        tag=("big2m" if kcn == HC else "w4"),
                        name=f"w_{name}",
                    )
                    dma(t[:], dram[:])
                    tiles.append(t)
                else:
                    for h in range(kcn // HC):
                        t = stream.tile([P, HC, HC, P], BF16, tag="big2m",
                                        name=f"w_{name}_{h}")
                        dma(t[:], dram[:, h * HC:(h + 1) * HC])
                        tiles.append(t)
                return tiles

            def wsl(tiles, kc, oc):
                return tiles[kc // HC][:, kc % HC, oc, :]

            def mm_accum(ps, oc, specs):
                """specs: list of (tiles, kcn, rhs_sb). rhs_sb: [P, kcn, Bc]."""
                total = sum(kcn for _, kcn, _ in specs)
                i = 0
                for tiles, kcn, rhs in specs:
                    for kc in range(kcn):
                        nc.tensor.matmul(
                            out=ps[:],
                            lhsT=wsl(tiles, kc, oc),
                            rhs=rhs[:, kc, :],
                            start=(i == 0),
                            stop=(i == total - 1),
                        )
                        i += 1

            # ---------------- attention query q_b (+ combined bias) -------
            qb = const.tile([P, HC, Bc], F32)
            for oc in range(HC):
                ps = ps_g.tile([P, Bc], F32, tag="g")
                for kc in range(HC):
                    nc.tensor.matmul(
                        out=ps[:],
                        lhsT=wa_sb[:, kc, oc, :],
                        rhs=s_bf[:, kc, :],
                        start=(kc == 0),
                        stop=(kc == HC - 1),
                    )
                nc.vector.tensor_scalar_add(qb[:, oc, :], ps[:], batt_sb[:, oc:oc + 1])

            # ---------------- attention chunk ----------------
            a_sb = const.tile([P, CHC, Bc], F32)  # attended state a^T

            def emit_chunk(j):
                enc_j = enc_tiles[j]
                if j > 0:
                    nc.sync.dma_start(enc_j[:],
                                      enc_t[j].rearrange("c p n -> p c n"))
                vsc = ps_vsc.tile([1, CHUNK], F32, tag="vsc", name=f"vsc{j}")
                for hc in range(HC):
                    ps = ps_attn.tile([P, CHUNK], F32, tag="attn",
                                      name=f"attn{j}_{hc}")
                    for c in range(CHC):
                        nc.tensor.matmul(
                            out=ps[:],
                            lhsT=ua_sb[:, hc, c, :],
                            rhs=enc_j[:, c, :],
                            start=(c == 0),
                            stop=(c == CHC - 1),
                        )
                    th = scr.tile([P, CHUNK], BF16, tag="tanh", name=f"th{j}")
                    for bi in range(BPC):
                        b = BPC * j + bi
                        nc.scalar.activation(
                            out=th[:, bi * S:(bi + 1) * S],
                            in_=ps[:, bi * S:(bi + 1) * S],
                            func=AF.Tanh,
                            bias=qb[:, hc, b:b + 1],
                        )
                    nc.tensor.matmul(
                        out=vsc[:],
                        lhsT=va_sb[:, hc:hc + 1],
                        rhs=th[:],
                        start=(hc == 0),
                        stop=(hc == HC - 1),
                    )

                # softmax over t within each of the BPC b's of this chunk
                exp_row = scr.tile([1, CHUNK], F32, tag="exp", name=f"ex{j}")
                nc.scalar.activation(out=exp_row[:], in_=vsc[:], func=AF.Exp)
                den = scr.tile([1, BPC], F32, tag="den", name=f"den{j}")
                nc.vector.reduce_sum(
                    den[:], exp_row[:].rearrange("p (b t) -> p b t", t=S),
                    axis=mybir.AxisListType.X,
                )
                inv = scr.tile([1, BPC], F32, tag="inv", name=f"inv{j}")
                nc.vector.reciprocal(inv[:], den[:])
                pb512 = scr.tile([P, CHUNK], BF16, tag="pb", name=f"pb{j}")
                nc.vector.memset(pb512[:], 0.0)
                nc.vector.tensor_tensor(
                    out=pb512[0:1, :].rearrange("p (b t) -> p b t", t=S),
                    in0=exp_row[:].rearrange("p (b t) -> p b t", t=S),
                    in1=inv[:, :, None].to_broadcast([1, BPC, S]),
                    op=ALU.mult,
                )
                probs_f = pf_pool.tile([1, CHUNK], F32, tag="pf",
                                       name=f"pfl{j}")
                nc.vector.tensor_tensor(
                    out=probs_f[:].rearrange("p (b t) -> p b t", t=S),
                    in0=exp_row[:].rearrange("p (b t) -> p b t", t=S),
                    in1=inv[:, :, None].to_broadcast([1, BPC, S]),
                    op=ALU.mult,
                )
                nc.sync.dma_start(probs_o[j:j + 1, :], probs_f[:])

                # broadcast probs across partitions via ones-row matmul
                bc_ps = ps_bc.tile([P, CHUNK], F32, tag="bc", name=f"bc{j}")
                nc.tensor.matmul(
                    out=bc_ps[:], lhsT=ones_row[:], rhs=pb512[:],
                    start=True, stop=True,
                )
                pbc = scr.tile([P, CHUNK], BF16, tag="pbc", name=f"pbc{j}")
                nc.scalar.copy(out=pbc[:], in_=bc_ps[:])

                # attended sum contribution: a^T[ch, b] = sum_t p * enc
                # multiply on DVE; free-dim accumulation on the idle ACT
                # engine via activation(Copy, accum_out=...)
                for c in range(CHC):
                    prod = scr.tile([P, CHUNK], BF16, tag="prod",
                                    name=f"prod{j}_{c}")
                    nc.vector.tensor_mul(prod[:], enc_j[:, c, :], pbc[:])
                    for bi in range(BPC):
                        dump = scr.tile([P, S], BF16, tag="dump",
                                        name=f"du{j}_{c}_{bi}")
                        nc.scalar.activation(
                            out=dump[:],
                            in_=prod[:, bi * S:(bi + 1) * S],
                            func=AF.Copy,
                            accum_out=a_sb[:, c,
                                           BPC * j + bi:BPC * j + bi + 1],
                        )

            emit_chunk(0)
            if NCH > 1:
                emit_chunk(1)

            # ---- x^T from gathered embedding rows (PE transpose) ----
            x_f32 = const.tile([Bc, E], F32)
            nc.vector.tensor_copy(x_f32[:], x_nat[:])
            x_bf = const.tile([P, EC, Bc], BF16)
            for kc in range(EC):
                tp = ps_g.tile([P, Bc], F32, tag="g", name=f"xt{kc}")
                nc.tensor.transpose(
                    out=tp[:],
                    in_=x_f32[:, kc * P:(kc + 1) * P],
                    identity=ident[:Bc, :Bc],
                )
                nc.scalar.copy(out=x_bf[:, kc, :], in_=tp[:])

            for j in range(2, min(6, NCH)):
                emit_chunk(j)

            # x/s-only gate weights: sync queue, emitted after the enc chunks
            # so they never delay attention, but well before their matmuls
            w_wz = load_w("wz_x")
            w_uz = load_w("uz_s")
            w_wr = load_w("wr_x")
            w_ur = load_w("ur_s")
            w_w = load_w("w_x")
            w_vo = load_w("vo_x")

            # ------------- early GRU partials (z/r, x+s terms) -------------
            pz = const.tile([P, HC, Bc], F32)
            pr = const.tile([P, HC, Bc], F32)
            pst = const.tile([P, HC, Bc], F32)
            pt = const.tile([P, HC, Bc], F32)
            for oc in range(HC):
                ps = ps_g.tile([P, Bc], F32, tag="g")
                mm_accum(ps, oc, [(w_wz, EC, x_bf), (w_uz, HC, s_bf)])
                nc.scalar.copy(out=pz[:, oc, :], in_=ps[:])
                ps = ps_g.tile([P, Bc], F32, tag="g")
                mm_accum(ps, oc, [(w_wr, EC, x_bf), (w_ur, HC, s_bf)])
                nc.scalar.copy(out=pr[:, oc, :], in_=ps[:])

            for j in range(6, NCH):
                emit_chunk(j)

            # x-only partials fill the PE bubble while the last chunk's
            # softmax + attended-sum run on ACT/DVE
            for oc in range(HC):
                ps = ps_g.tile([P, Bc], F32, tag="g")
                mm_accum(ps, oc, [(w_w, EC, x_bf)])
                nc.scalar.copy(out=pst[:, oc, :], in_=ps[:])
                ps = ps_g.tile([P, Bc], F32, tag="g")
                mm_accum(ps, oc, [(w_vo, EC, x_bf)])
                nc.scalar.copy(out=pt[:, oc, :], in_=ps[:])

            # ---------------- GRU tail (a-dependent) ----------------
            a_bf = const.tile([P, CHC, Bc], BF16)
            nc.vector.tensor_copy(a_bf[:], a_sb[:])

            def gate(out_sb, partial, bias_col, func, specs):
                for oc in range(HC):
                    ps = ps_g.tile([P, Bc], F32, tag="g")
                    mm_accum(ps, oc, specs)
                    tmp = scr.tile([P, Bc], F32, tag="gtmp")
                    nc.vector.tensor_add(tmp[:], ps[:], partial[:, oc, :])
                    nc.scalar.activation(
                        out=out_sb[:, oc, :], in_=tmp[:], func=func,
                        bias=bias_col[:, oc:oc + 1],
                    )

            w_cz = load_w("cz_a")
            w_cr = load_w("cr_a")
            z_T = const.tile([P, HC, Bc], F32)
            gate(z_T, pz, bias_sb["z"], AF.Sigmoid, [(w_cz, CHC, a_bf)])

            r_T = const.tile([P, HC, Bc], F32)
            gate(r_T, pr, bias_sb["r"], AF.Sigmoid, [(w_cr, CHC, a_bf)])

            rs_bf = const.tile([P, HC, Bc], BF16)
            nc.vector.tensor_mul(rs_bf[:], r_T[:], s_sb[:])

            w_u = load_w("u_rs")
            w_c = load_w("c_a")
            st_T = const.tile([P, HC, Bc], F32)
            gate(st_T, pst, bias_sb["st"], AF.Tanh,
                 [(w_u, HC, rs_bf), (w_c, CHC, a_bf)])

            snew = const.tile([P, HC, Bc], F32)
            d_T = const.tile([P, HC, Bc], F32)
            nc.vector.tensor_tensor(d_T[:], st_T[:], s_sb[:], op=ALU.subtract)
            nc.vector.tensor_mul(snew[:], z_T[:], d_T[:])
            nc.vector.tensor_add(snew[:], snew[:], s_sb[:])
            nc.sync.dma_start(snew_o[:], snew[:])
            snew_bf = const.tile([P, HC, Bc], BF16)
            nc.vector.tensor_copy(snew_bf[:], snew[:])

            w_uo = load_w("uo_s")
            w_co = load_w("co_a")
            ti_T = const.tile([P, HC, Bc], F32)
            gate(ti_T, pt, bias_sb["t"], AF.Relu,
                 [(w_uo, HC, snew_bf), (w_co, CHC, a_bf)])
            nc.sync.dma_start(ti_o[:], ti_T[:])

    nc.compile()
    return nc


# =====================================================================
# host-side sharding / layout
# =====================================================================

def _pack_w(w):
    """[din, dout] -> [128, din/128, dout/128, 128] bf16 lhsT tile layout."""
    dn, do = w.shape
    return np.ascontiguousarray(
        w.astype(nbf).reshape(dn // P, P, do // P, P).transpose(1, 0, 2, 3)
    )

def _col(v):
    """[H] -> [128, HC] (p, hc) layout."""
    return np.ascontiguousarray(np.asarray(v, np.float32).reshape(-1, P).T)


def prep_in_maps(in_word, encoded_states, last_hid_state, emb, params,
                 Bc=16, ncores=NCORES):
    in_word = np.asarray(in_word)
    enc = np.asarray(encoded_states, np.float32)
    s_full = np.asarray(last_hid_state, np.float32)
    emb = np.asarray(emb, np.float32)
    Pm = {k: {kk: np.asarray(vv, np.float32) for kk, vv in v.items()}
          for k, v in params.items()}

    emb_bf = emb.astype(nbf)
    enc_bf = enc.astype(nbf)  # [S, B, CH]

    shared = {
        "ua_t": np.ascontiguousarray(
            Pm["u_a"]["w"].astype(nbf).reshape(CHC, P, HC, P)
            .transpose(2, 1, 0, 3)),
        "wa_t": _pack_w(Pm["w_a"]["w"]),
        "va_t": np.ascontiguousarray(
            Pm["v_a"]["w"].astype(nbf).reshape(HC, P).T),
        "batt_t": _col(Pm["w_a"]["b"] + Pm["u_a"]["b"]),
        "emb_t": emb_bf,
        "wz_x": _pack_w(Pm["w_z"]["w"]), "wr_x": _pack_w(Pm["w_r"]["w"]),
        "w_x": _pack_w(Pm["w"]["w"]), "vo_x": _pack_w(Pm["v_o"]["w"]),
        "uz_s": _pack_w(Pm["u_z"]["w"]), "ur_s": _pack_w(Pm["u_r"]["w"]),
        "u_rs": _pack_w(Pm["u"]["w"]), "uo_s": _pack_w(Pm["u_o"]["w"]),
        "cz_a": _pack_w(Pm["c_z"]["w"]), "cr_a": _pack_w(Pm["c_r"]["w"]),
        "c_a": _pack_w(Pm["c"]["w"]), "co_a": _pack_w(Pm["c_o"]["w"]),
        "bz_t": _col(Pm["w_z"]["b"] + Pm["u_z"]["b"] + Pm["c_z"]["b"]),
        "br_t": _col(Pm["w_r"]["b"] + Pm["u_r"]["b"] + Pm["c_r"]["b"]),
        "bst_t": _col(Pm["w"]["b"] + Pm["u"]["b"] + Pm["c"]["b"]),
        "bt_t": _col(Pm["u_o"]["b"] + Pm["v_o"]["b"] + Pm["c_o"]["b"]),
    }

    NCH = (Bc * S) // CHUNK
    in_maps = []
    for c in range(ncores):
        sl = slice(c * Bc, (c + 1) * Bc)
        ec = enc_bf[:, sl, :].transpose(2, 1, 0)      # [CH, Bc, S]
        ec = ec.reshape(CH, Bc * S)                    # n = b*S + t
        ec = ec.reshape(CHC, P, NCH, CHUNK).transpose(2, 0, 1, 3)
        m = dict(shared)
        m["enc_t"] = np.ascontiguousarray(ec)
        st = s_full[sl].T.reshape(HC, P, Bc).transpose(1, 0, 2)
        m["s_t"] = np.ascontiguousarray(st)
        m["iw_t"] = np.ascontiguousarray(
            in_word[sl].astype(np.int32).reshape(Bc, 1))
        in_maps.append(m)
    return in_maps


def assemble_outputs(results, Bc=16, ncores=NCORES):
    Bfull = Bc * ncores
    s_new = np.zeros((Bfull, H), np.float32)
    t_i = np.zeros((Bfull, H), np.float32)
    probs = np.zeros((S, Bfull, 1), np.float32)
    for c, r in enumerate(results):
        sl = slice(c * Bc, (c + 1) * Bc)
        s_new[sl] = r["snew_o"].transpose(2, 1, 0).reshape(Bc, H)
        t_i[sl] = r["ti_o"].transpose(2, 1, 0).reshape(Bc, H)
        probs[:, sl, 0] = r["probs_o"].reshape(Bc, S).T
    return s_new, t_i, probs


_NC_CACHE = {}


def _get_nc(Bc=16, V=V_FULL):
    key = (Bc, V)
    if key not in _NC_CACHE:
        _NC_CACHE[key] = build_decoder_nc(Bc=Bc, V=V)
    return _NC_CACHE[key]


def kernel(in_word, encoded_states, last_hid_state, emb, params):
    nc = _get_nc()
    in_maps = prep_in_maps(in_word, encoded_states, last_hid_state, emb, params)
    res = run_bass_kernel_spmd(nc, in_maps, core_ids=list(range(NCORES)))
    return assemble_outputs(res.results)


# revision 26
# speedup vs baseline: 1.2670x; 1.2670x over previous
"""Trainium2 Bass kernel for nn_Decoder (attention-conditioned GRU decoder step).

Strategy (data-parallel over batch B=128 across 8 cores, Bc=16 per core):
  - Host re-layouts inputs per core (transpose/cast only; all FLOPs on device):
      enc   -> enc_T tiles [n-chunk, ch-chunk, 128, 512] bf16 where the flat
               free index n = b*S + t (b-major) so softmax-over-t is chunk-local
      weights -> lhsT tile layout [128(p=k-in), kc, oc, 128(m)] bf16
  - Device per core:
      pass 1: pre^T[h, n] = u_a^T @ enc_T  (PE, bf16, fp32 PSUM)
              tanh(pre + q_b + bias) via ACT with per-partition bias
              scores = v_a . tanh  via PE (lhsT = v_a column)
              softmax per b (chunk-local, ACT exp + DVE reduce/reciprocal)
      pass 2: probs broadcast to 128 partitions via PE ones-row matmul,
              attended a^T[ch, b] via DVE multiply + segmented reduce
      GRU:    all gate matmuls as out^T[dout, b] = W^T(k-tiles) @ rhs^T,
              x/s-only partial sums run early (hidden under attention),
              a-dependent tail after pass 2. Pointwise on DVE, acts on ACT.
  - Embedding lookup on device via gpsimd indirect DMA gather.
"""

import os

os.environ.setdefault("MYCRO_LOCAL_CACHE", "1")

import numpy as np
import ml_dtypes

import concourse.bass as bass
import concourse.mybir as mybir
import concourse.tile as tile
from concourse import bacc
from concourse.bass_utils import run_bass_kernel_spmd
from concourse.masks import make_identity

BF16 = mybir.dt.bfloat16
F32 = mybir.dt.float32
I32 = mybir.dt.int32
nbf = ml_dtypes.bfloat16
AF = mybir.ActivationFunctionType
ALU = mybir.AluOpType

NCORES = 8
# full-problem dims (hardcoded per task contract)
V_FULL, E, H, S, B_FULL = 50000, 512, 1024, 256, 128
CH = 2 * H
P = 128
CHC = CH // P   # 16 ch chunks
HC = H // P     # 8 h chunks
EC = E // P     # 4 e chunks
CHUNK = 512     # free-dim chunk of n = b*S + t  (2 b's per chunk)
BPC = CHUNK // S  # b's per chunk = 2

GATE_SPECS = [
    # name, kc count, rhs source key
    ("wz_x", EC, "x"), ("wr_x", EC, "x"), ("w_x", EC, "x"), ("vo_x", EC, "x"),
    ("uz_s", HC, "s"), ("ur_s", HC, "s"),
    ("u_rs", HC, "rs"), ("uo_s", HC, "snew"),
    ("cz_a", CHC, "a"), ("cr_a", CHC, "a"), ("c_a", CHC, "a"), ("co_a", CHC, "a"),
]


def build_decoder_nc(Bc=16, V=V_FULL, enable_asserts=False):
    """Build + compile the per-core SPMD kernel. Returns the bacc object."""
    NCH = (Bc * S) // CHUNK  # n-chunks
    nc = bacc.Bacc(
        "TRN2", target_bir_lowering=False, debug=False, enable_asserts=enable_asserts
    )

    def din(name, shape, ty):
        return nc.dram_tensor(name, shape, ty, kind="ExternalInput").ap()

    def dout(name, shape, ty):
        return nc.dram_tensor(name, shape, ty, kind="ExternalOutput").ap()

    enc_t = din("enc_t", [NCH, CHC, P, CHUNK], BF16)
    ua_t = din("ua_t", [HC, P, CHC, P], BF16)
    wa_t = din("wa_t", [P, HC, HC, P], BF16)
    va_t = din("va_t", [P, HC], BF16)
    batt_t = din("batt_t", [P, HC], F32)
    s_t = din("s_t", [P, HC, Bc], F32)
    emb_t = din("emb_t", [V, E], BF16)
    iw_t = din("iw_t", [Bc, 1], I32)
    wts = {name: din(name, [P, kc, HC, P], BF16) for name, kc, _ in GATE_SPECS}
    bz_t = din("bz_t", [P, HC], F32)
    br_t = din("br_t", [P, HC], F32)
    bst_t = din("bst_t", [P, HC], F32)
    bt_t = din("bt_t", [P, HC], F32)

    probs_o = dout("probs_o", [NCH, CHUNK], F32)
    snew_o = dout("snew_o", [P, HC, Bc], F32)
    ti_o = dout("ti_o", [P, HC, Bc], F32)

    with tile.TileContext(nc) as tc:
        with (
            tc.tile_pool(name="const", bufs=1) as const,
            tc.tile_pool(name="stream", bufs=7) as stream,
            tc.tile_pool(name="stream4", bufs=4) as stream4,
            tc.tile_pool(name="scr", bufs=2) as scr,
            tc.tile_pool(name="pf", bufs=3) as pf_pool,
            tc.tile_pool(name="ps_attn", bufs=3, space="PSUM") as ps_attn,
            tc.tile_pool(name="ps_vsc", bufs=2, space="PSUM") as ps_vsc,
            tc.tile_pool(name="ps_bc", bufs=1, space="PSUM") as ps_bc,
            tc.tile_pool(name="ps_g", bufs=2, space="PSUM") as ps_g,
        ):
            # ---------------- constants / small inputs ----------------
            ident = const.tile([P, P], F32)
            make_identity(nc, ident[:])
            ones_row = const.tile([P, P], BF16)
            nc.vector.memset(ones_row[:], 0.0)
            nc.vector.memset(ones_row[0:1, :], 1.0)

            # sync (HWDGE) queue order is arrival-deadline order: iw (gather
            # gate), wa (q gate), ua+enc0 (first attention chunk), consts,
            # remaining enc chunks, then the a-dependent tail weights.
            iw_sb = const.tile([Bc, 1], I32)
            nc.sync.dma_start(iw_sb[:], iw_t[:])
            s_sb = const.tile([P, HC, Bc], F32)
            nc.sync.dma_start(s_sb[:], s_t[:])
            wa_sb = stream.tile([P, HC, HC, P], BF16, tag="big2m")
            nc.sync.dma_start(wa_sb[:], wa_t[:])
            batt_sb = const.tile([P, HC], F32)
            nc.sync.dma_start(batt_sb[:], batt_t[:])
            # u_a loads per-hc so chunk0/hc0 can start after only 1/8 of it
            ua_sb = const.tile([P, HC, CHC, P], BF16)
            nc.sync.dma_start(ua_sb[:, 0], ua_t[0])

            enc_tiles = []
            for j in range(NCH):
                ej = stream.tile([P, CHC, CHUNK], BF16, tag="big2m",
                                 name=f"enc_{j}")
                enc_tiles.append(ej)
            nc.sync.dma_start(enc_tiles[0][:],
                              enc_t[0].rearrange("c p n -> p c n"))
            for hc in range(1, HC):
                nc.sync.dma_start(ua_sb[:, hc], ua_t[hc])

            va_sb = const.tile([P, HC], BF16)
            nc.sync.dma_start(va_sb[:], va_t[:])
            bias_sb = {}
            for nm, t in (("z", bz_t), ("r", br_t), ("st", bst_t), ("t", bt_t)):
                bias_sb[nm] = const.tile([P, HC], F32, name=f"bias_{nm}")
                nc.sync.dma_start(bias_sb[nm][:], t[:])

            s_bf = const.tile([P, HC, Bc], BF16)
            nc.vector.tensor_copy(s_bf[:], s_sb[:])

            # embedding gather rides the gpsimd dynamic queue
            x_nat = const.tile([Bc, E], BF16)
            nc.gpsimd.indirect_dma_start(
                out=x_nat[:],
                out_offset=None,
                in_=emb_t[:],
                in_offset=bass.IndirectOffsetOnAxis(ap=iw_sb[:, :1], axis=0),
            )

            # ---------------- weight stream helpers ----------------
            def load_w(name, queue=None):
                kcn = dict((n, k) for n, k, _ in GATE_SPECS)[name]
                dram = wts[name]
                dma = (queue or nc.sync).dma_start
                tiles = []
                if kcn <= HC:
                    t = (stream if kcn == HC else stream4).tile(
                        [P, kcn, HC, P], BF16,
                        tag=("big2m" if kcn == HC else "w4"),
                        name=f"w_{name}",
                    )
                    dma(t[:], dram[:])
                    tiles.append(t)
                else:
                    for h in range(kcn // HC):
                        t = stream.tile([P, HC, HC, P], BF16, tag="big2m",
                                        name=f"w_{name}_{h}")
                        dma(t[:], dram[:, h * HC:(h + 1) * HC])
                        tiles.append(t)
                return tiles

            def wsl(tiles, kc, oc):
                return tiles[kc // HC][:, kc % HC, oc, :]

            def mm_accum(ps, oc, specs):
                """specs: list of (tiles, kcn, rhs_sb). rhs_sb: [P, kcn, Bc]."""
                total = sum(kcn for _, kcn, _ in specs)
                i = 0
                for tiles, kcn, rhs in specs:
                    for kc in range(kcn):
                        nc.tensor.matmul(
                            out=ps[:],
                            lhsT=wsl(tiles, kc, oc),
                            rhs=rhs[:, kc, :],
                            start=(i == 0),
                            stop=(i == total - 1),
                        )
                        i += 1

            # ---------------- attention query q_b (+ combined bias) -------
            qb = const.tile([P, HC, Bc], F32)
            for oc in range(HC):
                ps = ps_g.tile([P, Bc], F32, tag="g")
                for kc in range(HC):
                    nc.tensor.matmul(
                        out=ps[:],
                        lhsT=wa_sb[:, kc, oc, :],
                        rhs=s_bf[:, kc, :],
                        start=(kc == 0),
                        stop=(kc == HC - 1),
                    )
                nc.vector.tensor_scalar_add(qb[:, oc, :], ps[:], batt_sb[:, oc:oc + 1])

            # ---------------- attention chunk ----------------
            a_sb = const.tile([P, CHC, Bc], F32)  # attended state a^T

            def emit_chunk(j):
                enc_j = enc_tiles[j]
                if j > 0:
                    nc.sync.dma_start(enc_j[:],
                                      enc_t[j].rearrange("c p n -> p c n"))
                vsc = ps_vsc.tile([1, CHUNK], F32, tag="vsc", name=f"vsc{j}")
                for hc in range(HC):
                    ps = ps_attn.tile([P, CHUNK], F32, tag="attn",
                                      name=f"attn{j}_{hc}")
                    for c in range(CHC):
                        nc.tensor.matmul(
                            out=ps[:],
                            lhsT=ua_sb[:, hc, c, :],
                            rhs=enc_j[:, c, :],
                            start=(c == 0),
                            stop=(c == CHC - 1),
                        )
                    th = scr.tile([P, CHUNK], BF16, tag="tanh", name=f"th{j}")
                    for bi in range(BPC):
                        b = BPC * j + bi
                        nc.scalar.activation(
                            out=th[:, bi * S:(bi + 1) * S],
                            in_=ps[:, bi * S:(bi + 1) * S],
                            func=AF.Tanh,
                            bias=qb[:, hc, b:b + 1],
                        )
                    nc.tensor.matmul(
                        out=vsc[:],
                        lhsT=va_sb[:, hc:hc + 1],
                        rhs=th[:],
                        start=(hc == 0),
                        stop=(hc == HC - 1),
                    )

                # softmax over t within each of the BPC b's of this chunk
                exp_row = scr.tile([1, CHUNK], F32, tag="exp", name=f"ex{j}")
                nc.scalar.activation(out=exp_row[:], in_=vsc[:], func=AF.Exp)
                den = scr.tile([1, BPC], F32, tag="den", name=f"den{j}")
                nc.vector.reduce_sum(
                    den[:], exp_row[:].rearrange("p (b t) -> p b t", t=S),
                    axis=mybir.AxisListType.X,
                )
                inv = scr.tile([1, BPC], F32, tag="inv", name=f"inv{j}")
                nc.vector.reciprocal(inv[:], den[:])
                pb512 = scr.tile([P, CHUNK], BF16, tag="pb", name=f"pb{j}")
                nc.vector.memset(pb512[:], 0.0)
                nc.vector.tensor_tensor(
                    out=pb512[0:1, :].rearrange("p (b t) -> p b t", t=S),
                    in0=exp_row[:].rearrange("p (b t) -> p b t", t=S),
                    in1=inv[:, :, None].to_broadcast([1, BPC, S]),
                    op=ALU.mult,
                )
                probs_f = pf_pool.tile([1, CHUNK], F32, tag="pf",
                                       name=f"pfl{j}")
                nc.vector.tensor_tensor(
                    out=probs_f[:].rearrange("p (b t) -> p b t", t=S),
                    in0=exp_row[:].rearrange("p (b t) -> p b t", t=S),
                    in1=inv[:, :, None].to_broadcast([1, BPC, S]),
                    op=ALU.mult,
                )
                nc.sync.dma_start(probs_o[j:j + 1, :], probs_f[:])

                # broadcast probs across partitions via ones-row matmul
                bc_ps = ps_bc.tile([P, CHUNK], F32, tag="bc", name=f"bc{j}")
                nc.tensor.matmul(
                    out=bc_ps[:], lhsT=ones_row[:], rhs=pb512[:],
                    start=True, stop=True,
                )
                pbc = scr.tile([P, CHUNK], BF16, tag="pbc", name=f"pbc{j}")
                nc.scalar.copy(out=pbc[:], in_=bc_ps[:])

                # attended sum contribution: a^T[ch, b] = sum_t p * enc
                for c in range(CHC):
                    prod = scr.tile([P, CHUNK], BF16, tag="prod",
                                    name=f"prod{j}_{c}")
                    nc.vector.tensor_mul(prod[:], enc_j[:, c, :], pbc[:])
                    nc.vector.reduce_sum(
                        a_sb[:, c, BPC * j:BPC * (j + 1)],
                        prod[:].rearrange("p (b t) -> p b t", t=S),
                        axis=mybir.AxisListType.X,
                    )

            emit_chunk(0)
            if NCH > 1:
                emit_chunk(1)

            # ---- x^T from gathered embedding rows (PE transpose) ----
            x_f32 = const.tile([Bc, E], F32)
            nc.vector.tensor_copy(x_f32[:], x_nat[:])
            x_bf = const.tile([P, EC, Bc], BF16)
            for kc in range(EC):
                tp = ps_g.tile([P, Bc], F32, tag="g", name=f"xt{kc}")
                nc.tensor.transpose(
                    out=tp[:],
                    in_=x_f32[:, kc * P:(kc + 1) * P],
                    identity=ident[:Bc, :Bc],
                )
                nc.scalar.copy(out=x_bf[:, kc, :], in_=tp[:])

            for j in range(2, min(6, NCH)):
                emit_chunk(j)

            # x/s-only gate weights: sync queue, emitted after the enc chunks
            # so they never delay attention, but well before their matmuls
            w_wz = load_w("wz_x")
            w_uz = load_w("uz_s")
            w_wr = load_w("wr_x")
            w_ur = load_w("ur_s")
            w_w = load_w("w_x")
            w_vo = load_w("vo_x")

            # ------------- early GRU partials (z/r, x+s terms) -------------
            pz = const.tile([P, HC, Bc], F32)
            pr = const.tile([P, HC, Bc], F32)
            pst = const.tile([P, HC, Bc], F32)
            pt = const.tile([P, HC, Bc], F32)
            for oc in range(HC):
                ps = ps_g.tile([P, Bc], F32, tag="g")
                mm_accum(ps, oc, [(w_wz, EC, x_bf), (w_uz, HC, s_bf)])
                nc.scalar.copy(out=pz[:, oc, :], in_=ps[:])
                ps = ps_g.tile([P, Bc], F32, tag="g")
                mm_accum(ps, oc, [(w_wr, EC, x_bf), (w_ur, HC, s_bf)])
                nc.scalar.copy(out=pr[:, oc, :], in_=ps[:])

            for j in range(6, NCH):
                emit_chunk(j)

            # x-only partials fill the PE bubble while the last chunk's
            # softmax + attended-sum run on ACT/DVE
            for oc in range(HC):
                ps = ps_g.tile([P, Bc], F32, tag="g")
                mm_accum(ps, oc, [(w_w, EC, x_bf)])
                nc.scalar.copy(out=pst[:, oc, :], in_=ps[:])
                ps = ps_g.tile([P, Bc], F32, tag="g")
                mm_accum(ps, oc, [(w_vo, EC, x_bf)])
                nc.scalar.copy(out=pt[:, oc, :], in_=ps[:])

            # ---------------- GRU tail (a-dependent) ----------------
            a_bf = const.tile([P, CHC, Bc], BF16)
            nc.vector.tensor_copy(a_bf[:], a_sb[:])

            def gate(out_sb, partial, bias_col, func, specs):
                for oc in range(HC):
                    ps = ps_g.tile([P, Bc], F32, tag="g")
                    mm_accum(ps, oc, specs)
                    tmp = scr.tile([P, Bc], F32, tag="gtmp")
                    nc.vector.tensor_add(tmp[:], ps[:], partial[:, oc, :])
                    nc.scalar.activation(
                        out=out_sb[:, oc, :], in_=tmp[:], func=func,
                        bias=bias_col[:, oc:oc + 1],
                    )

            w_cz = load_w("cz_a")
            w_cr = load_w("cr_a")
            z_T = const.tile([P, HC, Bc], F32)
            gate(z_T, pz, bias_sb["z"], AF.Sigmoid, [(w_cz, CHC, a_bf)])

            r_T = const.tile([P, HC, Bc], F32)
            gate(r_T, pr, bias_sb["r"], AF.Sigmoid, [(w_cr, CHC, a_bf)])

            rs_bf = const.tile([P, HC, Bc], BF16)
            nc.vector.tensor_mul(rs_bf[:], r_T[:], s_sb[:])

            w_u = load_w("u_rs")
            w_c = load_w("c_a")
            st_T = const.tile([P, HC, Bc], F32)
            gate(st_T, pst, bias_sb["st"], AF.Tanh,
                 [(w_u, HC, rs_bf), (w_c, CHC, a_bf)])

            snew = const.tile([P, HC, Bc], F32)
            d_T = const.tile([P, HC, Bc], F32)
            nc.vector.tensor_tensor(d_T[:], st_T[:], s_sb[:], op=ALU.subtract)
            nc.vector.tensor_mul(snew[:], z_T[:], d_T[:])
            nc.vector.tensor_add(snew[:], snew[:], s_sb[:])
            nc.sync.dma_start(snew_o[:], snew[:])
            snew_bf = const.tile([P, HC, Bc], BF16)
            nc.vector.tensor_copy(snew_bf[:], snew[:])

            w_uo = load_w("uo_s")
            w_co = load_w("co_a")
            ti_T = const.tile([P, HC, Bc], F32)
            gate(ti_T, pt, bias_sb["t"], AF.Relu,
                 [(w_uo, HC, snew_bf), (w_co, CHC, a_bf)])
            nc.sync.dma_start(ti_o[:], ti_T[:])

    nc.compile()
    return nc


# =====================================================================
# host-side sharding / layout
# =====================================================================

def _pack_w(w):
    """[din, dout] -> [128, din/128, dout/128, 128] bf16 lhsT tile layout."""
    dn, do = w.shape
    return np.ascontiguousarray(
        w.astype(nbf).reshape(dn // P, P, do // P, P).transpose(1, 0, 2, 3)
    )

def _col(v):
    """[H] -> [128, HC] (p, hc) layout."""
    return np.ascontiguousarray(np.asarray(v, np.float32).reshape(-1, P).T)


def prep_in_maps(in_word, encoded_states, last_hid_state, emb, params,
                 Bc=16, ncores=NCORES):
    in_word = np.asarray(in_word)
    enc = np.asarray(encoded_states, np.float32)
    s_full = np.asarray(last_hid_state, np.float32)
    emb = np.asarray(emb, np.float32)
    Pm = {k: {kk: np.asarray(vv, np.float32) for kk, vv in v.items()}
          for k, v in params.items()}

    emb_bf = emb.astype(nbf)
    enc_bf = enc.astype(nbf)  # [S, B, CH]

    shared = {
        "ua_t": np.ascontiguousarray(
            Pm["u_a"]["w"].astype(nbf).reshape(CHC, P, HC, P)
            .transpose(2, 1, 0, 3)),
        "wa_t": _pack_w(Pm["w_a"]["w"]),
        "va_t": np.ascontiguousarray(
            Pm["v_a"]["w"].astype(nbf).reshape(HC, P).T),
        "batt_t": _col(Pm["w_a"]["b"] + Pm["u_a"]["b"]),
        "emb_t": emb_bf,
        "wz_x": _pack_w(Pm["w_z"]["w"]), "wr_x": _pack_w(Pm["w_r"]["w"]),
        "w_x": _pack_w(Pm["w"]["w"]), "vo_x": _pack_w(Pm["v_o"]["w"]),
        "uz_s": _pack_w(Pm["u_z"]["w"]), "ur_s": _pack_w(Pm["u_r"]["w"]),
        "u_rs": _pack_w(Pm["u"]["w"]), "uo_s": _pack_w(Pm["u_o"]["w"]),
        "cz_a": _pack_w(Pm["c_z"]["w"]), "cr_a": _pack_w(Pm["c_r"]["w"]),
        "c_a": _pack_w(Pm["c"]["w"]), "co_a": _pack_w(Pm["c_o"]["w"]),
        "bz_t": _col(Pm["w_z"]["b"] + Pm["u_z"]["b"] + Pm["c_z"]["b"]),
        "br_t": _col(Pm["w_r"]["b"] + Pm["u_r"]["b"] + Pm["c_r"]["b"]),
        "bst_t": _col(Pm["w"]["b"] + Pm["u"]["b"] + Pm["c"]["b"]),
        "bt_t": _col(Pm["u_o"]["b"] + Pm["v_o"]["b"] + Pm["c_o"]["b"]),
    }

    NCH = (Bc * S) // CHUNK
    in_maps = []
    for c in range(ncores):
        sl = slice(c * Bc, (c + 1) * Bc)
        ec = enc_bf[:, sl, :].transpose(2, 1, 0)      # [CH, Bc, S]
        ec = ec.reshape(CH, Bc * S)                    # n = b*S + t
        ec = ec.reshape(CHC, P, NCH, CHUNK).transpose(2, 0, 1, 3)
        m = dict(shared)
        m["enc_t"] = np.ascontiguousarray(ec)
        st = s_full[sl].T.reshape(HC, P, Bc).transpose(1, 0, 2)
        m["s_t"] = np.ascontiguousarray(st)
        m["iw_t"] = np.ascontiguousarray(
            in_word[sl].astype(np.int32).reshape(Bc, 1))
        in_maps.append(m)
    return in_maps


def assemble_outputs(results, Bc=16, ncores=NCORES):
    Bfull = Bc * ncores
    s_new = np.zeros((Bfull, H), np.float32)
    t_i = np.zeros((Bfull, H), np.float32)
    probs = np.zeros((S, Bfull, 1), np.float32)
    for c, r in enumerate(results):
        sl = slice(c * Bc, (c + 1) * Bc)
        s_new[sl] = r["snew_o"].transpose(2, 1, 0).reshape(Bc, H)
        t_i[sl] = r["ti_o"].transpose(2, 1, 0).reshape(Bc, H)
        probs[:, sl, 0] = r["probs_o"].reshape(Bc, S).T
    return s_new, t_i, probs


_NC_CACHE = {}


def _get_nc(Bc=16, V=V_FULL):
    key = (Bc, V)
    if key not in _NC_CACHE:
        _NC_CACHE[key] = build_decoder_nc(Bc=Bc, V=V)
    return _NC_CACHE[key]


def kernel(in_word, encoded_states, last_hid_state, emb, params):
    nc = _get_nc()
    in_maps = prep_in_maps(in_word, encoded_states, last_hid_state, emb, params)
    res = run_bass_kernel_spmd(nc, in_maps, core_ids=list(range(NCORES)))
    return assemble_outputs(res.results)
